# revision 22
# baseline (speedup 1.0000x reference)
"""Trainium2 Bass kernel for nn_CMix_x060moe (RWKV CMix + hash-routed MoE).

Strategy: expert-sharded SPMD over 8 NeuronCores. Hash routing depends only
on token_ids, so the host computes the token->expert assignment as part of
sharding: core e receives exactly 2048 tokens (expert e's kept tokens in
FIFO order, padded with capacity-dropped tokens from anywhere). Each core
computes the dense squared-ReLU FFN, its own expert's FFN and the sigmoid
receptance for its 2048 tokens; the host scatters rows back. No collectives
needed and the load is perfectly balanced.

The token shift (xk/xr) is affine in the inputs and is folded into the host
dispatch: the device receives xk, a pre-masked expert copy of xk, and xr
directly (bf16), so no element-wise front-log ever starves the PE. Weights
are bf16 (full PE rate, half the HBM traffic of f32). All 48 first-layer
output tiles (32 dense + 16 expert) are held in SBUF as bf16 so the entire
second layer accumulates in PSUM - there are no vector-engine accumulation
adds at all. Per output m-tile the receptance matmul chain is interleaved
so the sigmoid overlaps the 48-matmul accumulation chain and y is produced
straight from PSUM.

All activations live C-major ([C, tokens]) on device so every matmul keeps
weights as the stationary operand.
"""

import os

import ml_dtypes
import numpy as np

import concourse.mybir as mybir
import concourse.tile as tile
from concourse import bacc
from concourse.bass_utils import run_bass_kernel_spmd

LAST_RESULTS = None  # set on every kernel() call; holds BassKernelResults

B, T, C = 8, 2048, 1024
DFF, DFFE = 4096, 2048
E = 8
HASH_PRIME = 5099
CAP = (B * T) // E  # 2048
N = B * T

P = 128               # partitions
TB = 512              # matmul token width (psum bank)
SB = 1024             # super-block: tokens sharing one weight fetch
NBLK = CAP // SB      # 2
CT = C // P           # 8  C-tiles
MT_D = DFF // P       # 32 dense-hidden tiles
MT_E = DFFE // P      # 16 expert-hidden tiles
KT2 = MT_D + MT_E     # 48 second-layer contraction tiles (dense + expert)

F32 = mybir.dt.float32
BF16 = mybir.dt.bfloat16

_COMPILED = None


def _build():
    nc = bacc.Bacc(trn_type="TRN2")

    # xk/xr are packed [P, blk, ct, t] so a whole block is one DMA with
    # 16KB contiguous per-partition lines (the head is DMA-stream-bound)
    xk = nc.dram_tensor("xk", [P, NBLK * CT * SB], BF16, kind="ExternalInput")
    xkm = nc.dram_tensor("xkm", [CT, P, CAP], BF16, kind="ExternalInput")
    xr = nc.dram_tensor("xr", [P, NBLK * CT * SB], BF16, kind="ExternalInput")
    # weights, host-tiled p-major: w*[m][p][k*P+q] = W[k*P+p, m*P+q]
    wk = nc.dram_tensor("wk", [MT_D, P, CT * P], BF16, kind="ExternalInput")
    wek = nc.dram_tensor("wek", [MT_E, P, CT * P], BF16, kind="ExternalInput")
    # second layer: Wv (32 k-tiles) then Wev (16 k-tiles), concatenated
    w2 = nc.dram_tensor("w2", [CT, P, KT2 * P], BF16, kind="ExternalInput")
    wr = nc.dram_tensor("wr", [CT, P, CT * P], BF16, kind="ExternalInput")
    yout = nc.dram_tensor("y", [CT, P, CAP], F32, kind="ExternalOutput")

    with tile.TileContext(nc) as tc:
        with (
            tc.tile_pool(name="xin", bufs=2) as xin,
            tc.tile_pool(name="xmp", bufs=1) as xmp,
            tc.tile_pool(name="acts", bufs=1) as actp,
            tc.tile_pool(name="wfirst", bufs=3) as wfp,
            tc.tile_pool(name="wsecond", bufs=2) as wsp,
            tc.tile_pool(name="wrp", bufs=2) as wrp,
            tc.tile_pool(name="tmp", bufs=3) as tmpp,
            tc.tile_pool(name="outp", bufs=2) as outp,
            tc.tile_pool(name="warm", bufs=1) as warmp,
            tc.tile_pool(name="ps1", bufs=4, space="PSUM") as ps1,
            tc.tile_pool(name="ps2", bufs=3, space="PSUM") as ps2,
            tc.tile_pool(name="psr", bufs=1, space="PSUM") as psr,
        ):
            # PE warm-up: ~40 tiny matmuls on a zeroed tile keep the PE
            # busy through the HAM activity window (~3.4us) while the first
            # input/weight DMAs land, so real matmuls start at 2.4GHz.
            wu = warmp.tile([P, P], BF16, tag="wu", name="wu")
            nc.vector.memset(wu[:], 0.0)
            pw = ps1.tile([P, TB], F32, tag="ps1", name="pw")
            for _ in range(50):
                nc.tensor.matmul(pw[:, :P], wu[:], wu[:], start=True,
                                 stop=True, skip_group_check=True)

            for blk in range(NBLK):
                tok = slice(blk * SB, (blk + 1) * SB)
                toks = [slice(blk * SB + h * TB, blk * SB + (h + 1) * TB)
                        for h in range(2)]

                # ---- dense first layer: kt = relu(xk @ Wk)^2 ----
                # Priming: first two weight tiles, then the whole block's
                # xk as ONE dma (16KB/partition contiguous - ~1.7x the
                # descriptor efficiency of per-C-tile loads). sxm/sxr slice
                # DMAs are drip-fed inside the m loop so they never queue
                # ahead of the weight stream.
                # xk in 4 quarter-DMAs (4KB lines): the m=0 chain starts as
                # soon as the first C-tile pair lands instead of waiting for
                # the full 2MB. Issue order q0, wk0, q1..q3, wk1 matches the
                # m=0 chain's consumption order.
                sxk = xin.tile([P, CT * SB], BF16, tag="xall", name="sxk")
                QW = 2 * SB

                def xk_quarter(q):
                    nc.sync.dma_start(
                        sxk[:, q * QW:(q + 1) * QW],
                        xk[:, blk * CT * SB + q * QW:blk * CT * SB + (q + 1) * QW],
                    )

                xk_quarter(0)
                wt_pre = []
                for m in range(2):
                    wt = wfp.tile([P, CT * P], BF16, tag="w1", name="wt")
                    wt_pre.append(wt)
                nc.sync.dma_start(wt_pre[0][:], wk[0])
                for q in range(1, 4):
                    xk_quarter(q)
                nc.sync.dma_start(wt_pre[1][:], wk[1])
                sxm = [xmp.tile([P, SB], BF16, tag=f"xm{i}", name=f"sxm{i}")
                       for i in range(CT)]
                sxr = xin.tile([P, CT * SB], BF16, tag="xrall", name="sxr",
                               bufs=1)

                kt = [actp.tile([P, SB], BF16, tag=f"kt{i}", name=f"kt{i}", padded_shape=[P, SB + 32])
                      for i in range(MT_D)]
                for m in range(MT_D):
                    if m < 2:
                        wt = wt_pre[m]
                    else:
                        wt = wfp.tile([P, CT * P], BF16, tag="w1", name="wt")
                        nc.sync.dma_start(wt[:], wk[m])
                    if CT <= m < 2 * CT:  # drip-feed expert input
                        ct = m - CT
                        nc.sync.dma_start(sxm[ct][:], xkm[ct, :, tok])
                    elif 2 * CT <= m < 3 * CT:  # drip-feed receptance input
                        ct = m - 2 * CT
                        nc.sync.dma_start(
                            sxr[:, ct * SB:(ct + 1) * SB],
                            xr[:, (blk * CT + ct) * SB:(blk * CT + ct + 1) * SB],
                        )
                    for h in range(2):
                        ps = ps1.tile([P, TB], F32, tag="ps1", name="ps")
                        for k in range(CT):
                            nc.tensor.matmul(
                                ps[:], wt[:, k * P:(k + 1) * P],
                                sxk[:, k * SB + h * TB:k * SB + h * TB + TB],
                                start=(k == 0), stop=(k == CT - 1),
                            )
                        rl = tmpp.tile([P, TB], BF16, tag="rl", name="rl", bufs=2)
                        nc.vector.tensor_scalar_max(rl[:], ps[:], 0.0)
                        nc.scalar.square(kt[m][:, h * TB:(h + 1) * TB], rl[:])

                # ---- expert first layer on pre-masked input ----
                ht = [actp.tile([P, SB], BF16, tag=f"ht{i}", name=f"ht{i}", padded_shape=[P, SB + 32])
                      for i in range(MT_E)]
                for m in range(MT_E):
                    wt = wfp.tile([P, CT * P], BF16, tag="w1", name="wt")
                    nc.sync.dma_start(wt[:], wek[m])
                    for h in range(2):
                        ps = ps1.tile([P, TB], F32, tag="ps1", name="ps")
                        for k in range(CT):
                            nc.tensor.matmul(
                                ps[:], wt[:, k * P:(k + 1) * P],
                                sxm[k][:, h * TB:(h + 1) * TB],
                                start=(k == 0), stop=(k == CT - 1),
                            )
                        rl = tmpp.tile([P, TB], BF16, tag="rl", name="rl", bufs=2)
                        nc.vector.tensor_scalar_max(rl[:], ps[:], 0.0)
                        nc.scalar.square(ht[m][:, h * TB:(h + 1) * TB], rl[:])

                # ---- second layer + receptance, PSUM-resident kv ----
                for m in range(CT):
                    w2t = wsp.tile([P, KT2 * P], BF16, tag="w2", name="w2t")
                    nc.sync.dma_start(w2t[:], w2[m])
                    wrt = wrp.tile([P, CT * P], BF16, tag="wr", name="wrt")
                    nc.sync.dma_start(wrt[:], wr[m])
                    for h in range(2):
                        pr = psr.tile([P, TB], F32, tag="psr", name="pr")
                        for k in range(CT):
                            nc.tensor.matmul(
                                pr[:], wrt[:, k * P:(k + 1) * P],
                                sxr[:, k * SB + h * TB:k * SB + h * TB + TB],
                                start=(k == 0), stop=(k == CT - 1),
                            )
                        rm = tmpp.tile([P, TB], BF16, tag="rm", name="rm",
                                       bufs=2)
                        nc.scalar.activation(
                            rm[:], pr[:], mybir.ActivationFunctionType.Sigmoid
                        )
                        pv = ps2.tile([P, TB], F32, tag="ps2", name="pv")
                        for k in range(MT_D):
                            nc.tensor.matmul(
                                pv[:], w2t[:, k * P:(k + 1) * P],
                                kt[k][:, h * TB:(h + 1) * TB],
                                start=(k == 0), stop=False,
                            )
                        for k in range(MT_E):
                            nc.tensor.matmul(
                                pv[:], w2t[:, (MT_D + k) * P:(MT_D + k + 1) * P],
                                ht[k][:, h * TB:(h + 1) * TB],
                                start=False, stop=(k == MT_E - 1),
                            )
                        yo = outp.tile([P, TB], F32, tag="yo", name="yo")
                        nc.vector.tensor_tensor(
                            out=yo[:], in0=pv[:], in1=rm[:],
                            op=mybir.AluOpType.mult,
                        )
                        nc.sync.dma_start(yout[m, :, toks[h]], yo[:])

    nc.compile()
    return nc


def _routing(token_ids: np.ndarray):
    """Token -> (per-core global token list [E, CAP], per-core keep mask)."""
    tid = token_ids.reshape(N).astype(np.int64)
    eidx = (tid * HASH_PRIME) % E
    order = np.argsort(eidx, kind="stable")  # FIFO within expert
    counts = np.bincount(eidx, minlength=E)
    starts = np.zeros(E + 1, np.int64)
    np.cumsum(counts, out=starts[1:])

    token_lists = np.empty((E, CAP), np.int64)
    masks = np.zeros((E, CAP), np.float32)
    dropped = []
    fill_needed = []
    for e in range(E):
        grp = order[starts[e]:starts[e + 1]]
        nk = min(len(grp), CAP)
        token_lists[e, :nk] = grp[:nk]
        masks[e, :nk] = 1.0
        dropped.append(grp[CAP:])
        fill_needed.append(CAP - nk)
    dropped = (
        np.concatenate(dropped) if dropped else np.empty(0, np.int64)
    )
    pos = 0
    for e in range(E):
        need = fill_needed[e]
        if need:
            token_lists[e, CAP - need:] = dropped[pos:pos + need]
            pos += need
    assert pos == len(dropped)
    return token_lists, masks


def _tile_w(W, mt):
    """[C_in, M] -> [mt, P, kt*P] bf16 with w[m][p][k*P+q] = W[k*P+p, m*P+q]."""
    kt = W.shape[0] // P
    return np.ascontiguousarray(
        W.reshape(kt, P, mt, P).transpose(2, 1, 0, 3).reshape(mt, P, kt * P)
    ).astype(ml_dtypes.bfloat16)


def kernel(x, shift_state, token_ids, time_maa_k, time_maa_r, Wk, Wv, Wr, Wek, Wev):
    global _COMPILED
    if _COMPILED is None:
        _COMPILED = _build()
    nc = _COMPILED

    x = np.asarray(x, np.float32)
    shift_state = np.asarray(shift_state, np.float32)
    token_lists, masks = _routing(np.asarray(token_ids))

    xf = x.reshape(N, C)
    xprev_f = np.empty_like(xf)
    xprev_f[1:] = xf[:-1]
    xprev_f[np.arange(B) * T] = shift_state

    # token shift folded into dispatch (f32 exact, matches reference)
    maak = np.asarray(time_maa_k, np.float32)
    maar = np.asarray(time_maa_r, np.float32)
    dxf = xprev_f - xf
    xk_full = xf + dxf * maak
    xr_full = xf + dxf * maar

    wk_t = _tile_w(np.asarray(Wk, np.float32), MT_D)
    wr_t = _tile_w(np.asarray(Wr, np.float32), CT)
    Wv = np.asarray(Wv, np.float32)
    Wek = np.asarray(Wek, np.float32)
    Wev = np.asarray(Wev, np.float32)

    def ctmajor_bf16(rows):  # [CAP, C] -> [CT, P, CAP] bf16
        return np.ascontiguousarray(
            rows.T.reshape(CT, P, CAP)
        ).astype(ml_dtypes.bfloat16)

    def packed_bf16(rows):  # [CAP, C] -> [P, NBLK*CT*SB] bf16, [p][blk][ct][t]
        a = rows.T.reshape(CT, P, NBLK, SB).transpose(1, 2, 0, 3)
        return np.ascontiguousarray(
            a.reshape(P, NBLK * CT * SB)
        ).astype(ml_dtypes.bfloat16)

    in_maps = []
    for e in range(E):
        L = token_lists[e]
        xk_rows = xk_full[L]
        in_maps.append(dict(
            xk=packed_bf16(xk_rows),
            xkm=ctmajor_bf16(xk_rows * masks[e][:, None]),
            xr=packed_bf16(xr_full[L]),
            wk=wk_t,
            wek=_tile_w(Wek[e], MT_E),
            w2=_tile_w(np.concatenate([Wv, Wev[e]], axis=0), CT),
            wr=wr_t,
        ))

    res = run_bass_kernel_spmd(
        nc, in_maps, core_ids=list(range(E)),
        trace=bool(os.environ.get("KERNEL_TRACE")),
    )
    global LAST_RESULTS
    LAST_RESULTS = res

    y = np.empty((N, C), np.float32)
    for e in range(E):
        y[token_lists[e]] = res.results[e]["y"].reshape(C, CAP).T
    return y.reshape(B, T, C)


# revision 23
# speedup vs baseline: 1.0016x; 1.0016x over previous
"""Trainium2 Bass kernel for nn_CMix_x060moe (RWKV CMix + hash-routed MoE).

Strategy: expert-sharded SPMD over 8 NeuronCores. Hash routing depends only
on token_ids, so the host computes the token->expert assignment as part of
sharding: core e receives exactly 2048 tokens (expert e's kept tokens in
FIFO order, padded with capacity-dropped tokens from anywhere). Each core
computes the dense squared-ReLU FFN, its own expert's FFN and the sigmoid
receptance for its 2048 tokens; the host scatters rows back. No collectives
needed and the load is perfectly balanced.

The token shift (xk/xr) is affine in the inputs and is folded into the host
dispatch: the device receives xk, a pre-masked expert copy of xk, and xr
directly (bf16), so no element-wise front-log ever starves the PE. Weights
are bf16 (full PE rate, half the HBM traffic of f32). All 48 first-layer
output tiles (32 dense + 16 expert) are held in SBUF as bf16 so the entire
second layer accumulates in PSUM - there are no vector-engine accumulation
adds at all. Per output m-tile the receptance matmul chain is interleaved
so the sigmoid overlaps the 48-matmul accumulation chain and y is produced
straight from PSUM.

All activations live C-major ([C, tokens]) on device so every matmul keeps
weights as the stationary operand.
"""

import os

import ml_dtypes
import numpy as np

import concourse.mybir as mybir
import concourse.tile as tile
from concourse import bacc
from concourse.bass_utils import run_bass_kernel_spmd

LAST_RESULTS = None  # set on every kernel() call; holds BassKernelResults

B, T, C = 8, 2048, 1024
DFF, DFFE = 4096, 2048
E = 8
HASH_PRIME = 5099
CAP = (B * T) // E  # 2048
N = B * T

P = 128               # partitions
TB = 512              # matmul token width (psum bank)
SB = 1024             # super-block: tokens sharing one weight fetch
NBLK = CAP // SB      # 2
CT = C // P           # 8  C-tiles
MT_D = DFF // P       # 32 dense-hidden tiles
MT_E = DFFE // P      # 16 expert-hidden tiles
KT2 = MT_D + MT_E     # 48 second-layer contraction tiles (dense + expert)

F32 = mybir.dt.float32
BF16 = mybir.dt.bfloat16

_COMPILED = None


def _build():
    nc = bacc.Bacc(trn_type="TRN2")

    # xk/xr are packed [P, blk, ct, t] so a whole block is one DMA with
    # 16KB contiguous per-partition lines (the head is DMA-stream-bound)
    xk = nc.dram_tensor("xk", [P, NBLK * CT * SB], BF16, kind="ExternalInput")
    xkm = nc.dram_tensor("xkm", [CT, P, CAP], BF16, kind="ExternalInput")
    xr = nc.dram_tensor("xr", [P, NBLK * CT * SB], BF16, kind="ExternalInput")
    # weights, host-tiled p-major: w*[m][p][k*P+q] = W[k*P+p, m*P+q]
    wk = nc.dram_tensor("wk", [MT_D, P, CT * P], BF16, kind="ExternalInput")
    wek = nc.dram_tensor("wek", [MT_E, P, CT * P], BF16, kind="ExternalInput")
    # second layer: Wv (32 k-tiles) then Wev (16 k-tiles), concatenated
    w2 = nc.dram_tensor("w2", [CT, P, KT2 * P], BF16, kind="ExternalInput")
    wr = nc.dram_tensor("wr", [CT, P, CT * P], BF16, kind="ExternalInput")
    yout = nc.dram_tensor("y", [CT, P, CAP], F32, kind="ExternalOutput")

    with tile.TileContext(nc) as tc:
        with (
            tc.tile_pool(name="xin", bufs=2) as xin,
            tc.tile_pool(name="xmp", bufs=1) as xmp,
            tc.tile_pool(name="acts", bufs=1) as actp,
            tc.tile_pool(name="wfirst", bufs=4) as wfp,
            tc.tile_pool(name="wsecond", bufs=2) as wsp,
            tc.tile_pool(name="wrp", bufs=2) as wrp,
            tc.tile_pool(name="tmp", bufs=3) as tmpp,
            tc.tile_pool(name="outp", bufs=3) as outp,
            tc.tile_pool(name="warm", bufs=1) as warmp,
            tc.tile_pool(name="ps1", bufs=4, space="PSUM") as ps1,
            tc.tile_pool(name="ps2", bufs=3, space="PSUM") as ps2,
            tc.tile_pool(name="psr", bufs=1, space="PSUM") as psr,
        ):
            # PE warm-up: ~40 tiny matmuls on a zeroed tile keep the PE
            # busy through the HAM activity window (~3.4us) while the first
            # input/weight DMAs land, so real matmuls start at 2.4GHz.
            wu = warmp.tile([P, P], BF16, tag="wu", name="wu")
            nc.vector.memset(wu[:], 0.0)
            pw = ps1.tile([P, TB], F32, tag="ps1", name="pw")
            for _ in range(50):
                nc.tensor.matmul(pw[:, :P], wu[:], wu[:], start=True,
                                 stop=True, skip_group_check=True)

            for blk in range(NBLK):
                tok = slice(blk * SB, (blk + 1) * SB)
                toks = [slice(blk * SB + h * TB, blk * SB + (h + 1) * TB)
                        for h in range(2)]

                # ---- dense first layer: kt = relu(xk @ Wk)^2 ----
                # Priming: first two weight tiles, then the whole block's
                # xk as ONE dma (16KB/partition contiguous - ~1.7x the
                # descriptor efficiency of per-C-tile loads). sxm/sxr slice
                # DMAs are drip-fed inside the m loop so they never queue
                # ahead of the weight stream.
                # xk in 4 quarter-DMAs (4KB lines): the m=0 chain starts as
                # soon as the first C-tile pair lands instead of waiting for
                # the full 2MB. Issue order q0, wk0, q1..q3, wk1 matches the
                # m=0 chain's consumption order.
                sxk = xin.tile([P, CT * SB], BF16, tag="xall", name="sxk")
                QW = 2 * SB

                def xk_quarter(q):
                    nc.sync.dma_start(
                        sxk[:, q * QW:(q + 1) * QW],
                        xk[:, blk * CT * SB + q * QW:blk * CT * SB + (q + 1) * QW],
                    )

                xk_quarter(0)
                wt_pre = []
                for m in range(2):
                    wt = wfp.tile([P, CT * P], BF16, tag="w1", name="wt")
                    wt_pre.append(wt)
                nc.sync.dma_start(wt_pre[0][:], wk[0])
                for q in range(1, 4):
                    xk_quarter(q)
                nc.sync.dma_start(wt_pre[1][:], wk[1])
                sxm = [xmp.tile([P, SB], BF16, tag=f"xm{i}", name=f"sxm{i}")
                       for i in range(CT)]
                sxr = xin.tile([P, CT * SB], BF16, tag="xrall", name="sxr",
                               bufs=1)

                kt = [actp.tile([P, SB], BF16, tag=f"kt{i}", name=f"kt{i}")
                      for i in range(MT_D)]
                for m in range(MT_D):
                    if m < 2:
                        wt = wt_pre[m]
                    else:
                        wt = wfp.tile([P, CT * P], BF16, tag="w1", name="wt")
                        nc.sync.dma_start(wt[:], wk[m])
                    if CT <= m < 2 * CT:  # drip-feed expert input
                        ct = m - CT
                        nc.sync.dma_start(sxm[ct][:], xkm[ct, :, tok])
                    elif 2 * CT <= m < 3 * CT:  # drip-feed receptance input
                        ct = m - 2 * CT
                        nc.sync.dma_start(
                            sxr[:, ct * SB:(ct + 1) * SB],
                            xr[:, (blk * CT + ct) * SB:(blk * CT + ct + 1) * SB],
                        )
                    for h in range(2):
                        ps = ps1.tile([P, TB], F32, tag="ps1", name="ps")
                        for k in range(CT):
                            nc.tensor.matmul(
                                ps[:], wt[:, k * P:(k + 1) * P],
                                sxk[:, k * SB + h * TB:k * SB + h * TB + TB],
                                start=(k == 0), stop=(k == CT - 1),
                            )
                        rl = tmpp.tile([P, TB], BF16, tag="rl", name="rl")
                        nc.vector.tensor_scalar_max(rl[:], ps[:], 0.0)
                        nc.scalar.square(kt[m][:, h * TB:(h + 1) * TB], rl[:])

                # ---- expert first layer on pre-masked input ----
                ht = [actp.tile([P, SB], BF16, tag=f"ht{i}", name=f"ht{i}")
                      for i in range(MT_E)]
                for m in range(MT_E):
                    wt = wfp.tile([P, CT * P], BF16, tag="w1", name="wt")
                    nc.sync.dma_start(wt[:], wek[m])
                    for h in range(2):
                        ps = ps1.tile([P, TB], F32, tag="ps1", name="ps")
                        for k in range(CT):
                            nc.tensor.matmul(
                                ps[:], wt[:, k * P:(k + 1) * P],
                                sxm[k][:, h * TB:(h + 1) * TB],
                                start=(k == 0), stop=(k == CT - 1),
                            )
                        rl = tmpp.tile([P, TB], BF16, tag="rl", name="rl")
                        nc.vector.tensor_scalar_max(rl[:], ps[:], 0.0)
                        nc.scalar.square(ht[m][:, h * TB:(h + 1) * TB], rl[:])

                # ---- second layer + receptance, PSUM-resident kv ----
                for m in range(CT):
                    w2t = wsp.tile([P, KT2 * P], BF16, tag="w2", name="w2t")
                    nc.sync.dma_start(w2t[:], w2[m])
                    wrt = wrp.tile([P, CT * P], BF16, tag="wr", name="wrt")
                    nc.sync.dma_start(wrt[:], wr[m])
                    for h in range(2):
                        pr = psr.tile([P, TB], F32, tag="psr", name="pr")
                        for k in range(CT):
                            nc.tensor.matmul(
                                pr[:], wrt[:, k * P:(k + 1) * P],
                                sxr[:, k * SB + h * TB:k * SB + h * TB + TB],
                                start=(k == 0), stop=(k == CT - 1),
                            )
                        rm = tmpp.tile([P, TB], BF16, tag="rm", name="rm",
                                       bufs=2)
                        nc.scalar.activation(
                            rm[:], pr[:], mybir.ActivationFunctionType.Sigmoid
                        )
                        pv = ps2.tile([P, TB], F32, tag="ps2", name="pv")
                        for k in range(MT_D):
                            nc.tensor.matmul(
                                pv[:], w2t[:, k * P:(k + 1) * P],
                                kt[k][:, h * TB:(h + 1) * TB],
                                start=(k == 0), stop=False,
                            )
                        for k in range(MT_E):
                            nc.tensor.matmul(
                                pv[:], w2t[:, (MT_D + k) * P:(MT_D + k + 1) * P],
                                ht[k][:, h * TB:(h + 1) * TB],
                                start=False, stop=(k == MT_E - 1),
                            )
                        yo = outp.tile([P, TB], F32, tag="yo", name="yo")
                        nc.vector.tensor_tensor(
                            out=yo[:], in0=pv[:], in1=rm[:],
                            op=mybir.AluOpType.mult,
                        )
                        nc.sync.dma_start(yout[m, :, toks[h]], yo[:])

    nc.compile()
    return nc


def _routing(token_ids: np.ndarray):
    """Token -> (per-core global token list [E, CAP], per-core keep mask)."""
    tid = token_ids.reshape(N).astype(np.int64)
    eidx = (tid * HASH_PRIME) % E
    order = np.argsort(eidx, kind="stable")  # FIFO within expert
    counts = np.bincount(eidx, minlength=E)
    starts = np.zeros(E + 1, np.int64)
    np.cumsum(counts, out=starts[1:])

    token_lists = np.empty((E, CAP), np.int64)
    masks = np.zeros((E, CAP), np.float32)
    dropped = []
    fill_needed = []
    for e in range(E):
        grp = order[starts[e]:starts[e + 1]]
        nk = min(len(grp), CAP)
        token_lists[e, :nk] = grp[:nk]
        masks[e, :nk] = 1.0
        dropped.append(grp[CAP:])
        fill_needed.append(CAP - nk)
    dropped = (
        np.concatenate(dropped) if dropped else np.empty(0, np.int64)
    )
    pos = 0
    for e in range(E):
        need = fill_needed[e]
        if need:
            token_lists[e, CAP - need:] = dropped[pos:pos + need]
            pos += need
    assert pos == len(dropped)
    return token_lists, masks


def _tile_w(W, mt):
    """[C_in, M] -> [mt, P, kt*P] bf16 with w[m][p][k*P+q] = W[k*P+p, m*P+q]."""
    kt = W.shape[0] // P
    return np.ascontiguousarray(
        W.reshape(kt, P, mt, P).transpose(2, 1, 0, 3).reshape(mt, P, kt * P)
    ).astype(ml_dtypes.bfloat16)


def kernel(x, shift_state, token_ids, time_maa_k, time_maa_r, Wk, Wv, Wr, Wek, Wev):
    global _COMPILED
    if _COMPILED is None:
        _COMPILED = _build()
    nc = _COMPILED

    x = np.asarray(x, np.float32)
    shift_state = np.asarray(shift_state, np.float32)
    token_lists, masks = _routing(np.asarray(token_ids))

    xf = x.reshape(N, C)
    xprev_f = np.empty_like(xf)
    xprev_f[1:] = xf[:-1]
    xprev_f[np.arange(B) * T] = shift_state

    # token shift folded into dispatch (f32 exact, matches reference)
    maak = np.asarray(time_maa_k, np.float32)
    maar = np.asarray(time_maa_r, np.float32)
    dxf = xprev_f - xf
    xk_full = xf + dxf * maak
    xr_full = xf + dxf * maar

    wk_t = _tile_w(np.asarray(Wk, np.float32), MT_D)
    wr_t = _tile_w(np.asarray(Wr, np.float32), CT)
    Wv = np.asarray(Wv, np.float32)
    Wek = np.asarray(Wek, np.float32)
    Wev = np.asarray(Wev, np.float32)

    def ctmajor_bf16(rows):  # [CAP, C] -> [CT, P, CAP] bf16
        return np.ascontiguousarray(
            rows.T.reshape(CT, P, CAP)
        ).astype(ml_dtypes.bfloat16)

    def packed_bf16(rows):  # [CAP, C] -> [P, NBLK*CT*SB] bf16, [p][blk][ct][t]
        a = rows.T.reshape(CT, P, NBLK, SB).transpose(1, 2, 0, 3)
        return np.ascontiguousarray(
            a.reshape(P, NBLK * CT * SB)
        ).astype(ml_dtypes.bfloat16)

    in_maps = []
    for e in range(E):
        L = token_lists[e]
        xk_rows = xk_full[L]
        in_maps.append(dict(
            xk=packed_bf16(xk_rows),
            xkm=ctmajor_bf16(xk_rows * masks[e][:, None]),
            xr=packed_bf16(xr_full[L]),
            wk=wk_t,
            wek=_tile_w(Wek[e], MT_E),
            w2=_tile_w(np.concatenate([Wv, Wev[e]], axis=0), CT),
            wr=wr_t,
        ))

    res = run_bass_kernel_spmd(
        nc, in_maps, core_ids=list(range(E)),
        trace=bool(os.environ.get("KERNEL_TRACE")),
    )
    global LAST_RESULTS
    LAST_RESULTS = res

    y = np.empty((N, C), np.float32)
    for e in range(E):
        y[token_lists[e]] = res.results[e]["y"].reshape(C, CAP).T
    return y.reshape(B, T, C)
